# revision 6
# baseline (speedup 1.0000x reference)
# GNN message-passing kernel for Trainium2 (8 NeuronCores, SPMD).
#
# Computes: lam = exp(adj) / exp(adj).sum(axis=1, keepdims=True); out = lam @ h
#   adj: [16384, 16384] f32, h: [16384, 128] f32 -> out: [16384, 128] f32
#
# Sharding: adj row-sharded across 8 cores ([2048, 16384] each), h replicated.
# Per-core algorithm (all in one pass over adj, memory-bound by the 128 MiB
# adj slice):
#   - DMA adj chunk [128, W] f32 -> SBUF
#   - ScalarE: E = exp(chunk) cast to f16 (SBUF->SBUF)
#   - TensorE: transpose 128x128 blocks of E via identity matmul -> PSUM
#   - VectorE: copy PSUM -> SBUF (cast back to f16)
#   - TensorE: accumulate E_block^T.T @ h_aug into PSUM out tile, where
#     h_aug = [h | ones] f16 [128, 129]; column 128 accumulates the row sums
#     of exp(adj) so the softmax denominator comes free from the matmul.
#   - VectorE: out = psum[:, :128] * (1 / psum[:, 128]) -> DMA to HBM.

import os
import sys

for _p in ("/opt/trn_rl_repo", "/root/.axon_site/_ro/trn_rl_repo"):
    if os.path.isdir(_p) and _p not in sys.path:
        sys.path.append(_p)

import numpy as np

import concourse.bass as bass  # noqa: F401  (registers AP machinery)
import concourse.tile as tile
from concourse import bacc, mybir
from concourse.bass_utils import run_bass_kernel_spmd
from concourse.masks import make_identity

N_CORES = 8
N = 16384
D = 128
ROWS = N // N_CORES  # 2048 rows of adj per core
P = 128


def pack_h(h, d):
    """Host-side prep: [cols, d] f32 -> [128, cols//128, d+1] f32 with a ones
    column appended, laid out so h_aug[p, jc, :] = [h[jc*128 + p, :], 1.0].
    Contiguous in this layout so the device load is a single clean DMA."""
    cols = h.shape[0]
    njc = cols // P
    hp = h.reshape(njc, P, d).transpose(1, 0, 2)  # [p, jc, d]
    ones = np.ones((P, njc, 1), np.float32)
    return np.ascontiguousarray(np.concatenate([hp, ones], axis=2), dtype=np.float32)


def kernel_body(tc, out_ap, adj_ap, h_ap, rows, cols, d, w):
    """Tile kernel for one core: out = softmax_rows(adj) @ h.

    adj_ap: [rows, cols] f32 DRAM; h_ap: [128, cols//128, d+1] f32 DRAM
    (pre-packed by pack_h); out_ap: [rows, d] f32 DRAM.
    w: column chunk width (multiple of 128).
    """
    nc = tc.nc
    f32 = mybir.dt.float32
    f16 = mybir.dt.float16

    NT = rows // P  # row tiles
    CH = cols // w  # column chunks per row tile
    BPC = w // P  # 128-wide j-blocks per chunk
    GRP = min(8, BPC)  # j-blocks per PSUM staging group
    NG = BPC // GRP
    NJC = cols // P  # total j-blocks
    assert rows % P == 0 and cols % w == 0 and w % (GRP * P) == 0

    with (
        tc.tile_pool(name="singles", bufs=1) as singles,
        tc.tile_pool(name="adjp", bufs=3) as adjp,
        tc.tile_pool(name="ep", bufs=2) as ep,
        tc.tile_pool(name="etp", bufs=3) as etp,
        tc.tile_pool(name="outp", bufs=2) as outp,
        tc.tile_pool(name="rp", bufs=2) as rp,
        tc.tile_pool(name="psacc", bufs=2, space="PSUM") as psacc,
        tc.tile_pool(name="psstage", bufs=2, space="PSUM") as psstage,
    ):
        identity = singles.tile([P, P], f16)
        make_identity(nc, identity)

        # h_aug[p, jc, :] = [h[jc*128 + p, :], 1.0] in f16 (pre-packed on host)
        h_aug = singles.tile([P, NJC, d + 1], f16)
        nc.gpsimd.dma_start(out=h_aug[:], in_=h_ap)  # contiguous f32 -> f16 cast

        for it in range(NT):
            acc = psacc.tile([P, d + 1], f32)
            for c in range(CH):
                a = adjp.tile([P, w], f32)
                nc.sync.dma_start(a[:], adj_ap[it * P : (it + 1) * P, c * w : (c + 1) * w])
                e = ep.tile([P, w], f16)
                nc.scalar.activation(e[:], a[:], mybir.ActivationFunctionType.Exp)
                for g in range(NG):
                    pt = psstage.tile([P, GRP, P], f16)
                    for q in range(GRP):
                        jb = g * GRP + q
                        nc.tensor.transpose(
                            pt[:, q, :], e[:, jb * P : (jb + 1) * P], identity
                        )
                    et = etp.tile([P, GRP, P], f16)
                    nc.vector.tensor_copy(out=et[:], in_=pt[:])
                    for q in range(GRP):
                        jc = c * BPC + g * GRP + q
                        nc.tensor.matmul(
                            acc[:],
                            lhsT=et[:, q, :],
                            rhs=h_aug[:, jc, :],
                            start=(jc == 0),
                            stop=(jc == NJC - 1),
                        )
            rec = rp.tile([P, 1], f32)
            nc.vector.reciprocal(rec[:], acc[:, d : d + 1])
            ob = outp.tile([P, d], f32)
            nc.vector.tensor_scalar_mul(ob[:], acc[:, 0:d], rec[:])
            nc.sync.dma_start(out_ap[it * P : (it + 1) * P, :], ob[:])


def build_nc(rows=ROWS, cols=N, d=D, w=4096, num_devices=N_CORES):
    nc = bacc.Bacc(
        "TRN2", target_bir_lowering=False, debug=False, num_devices=num_devices
    )
    adj = nc.dram_tensor("adj", [rows, cols], mybir.dt.float32, kind="ExternalInput").ap()
    h = nc.dram_tensor(
        "h", [P, cols // P, d + 1], mybir.dt.float32, kind="ExternalInput"
    ).ap()
    out = nc.dram_tensor("out", [rows, d], mybir.dt.float32, kind="ExternalOutput").ap()
    with tile.TileContext(nc) as tc:
        kernel_body(tc, out, adj, h, rows, cols, d, w)
    nc.compile()
    return nc


_nc_cache = None


def kernel(h, adj):
    global _nc_cache
    if _nc_cache is None:
        _nc_cache = build_nc()
    nc = _nc_cache
    h = np.ascontiguousarray(np.asarray(h, dtype=np.float32))
    adj = np.ascontiguousarray(np.asarray(adj, dtype=np.float32))
    h_packed = pack_h(h, D)
    in_maps = [
        {"adj": adj[i * ROWS : (i + 1) * ROWS], "h": h_packed}
        for i in range(N_CORES)
    ]
    res = run_bass_kernel_spmd(nc, in_maps, core_ids=list(range(N_CORES)))
    return np.concatenate([r["out"] for r in res.results], axis=0)


# revision 13
# speedup vs baseline: 12.0481x; 12.0481x over previous
# GNN message-passing kernel for Trainium2 (8 NeuronCores, SPMD).
#
# Computes: lam = exp(adj) / exp(adj).sum(axis=1, keepdims=True); out = lam @ h
#   adj: [16384, 16384] f32, h: [16384, 128] f32 -> out: [16384, 128] f32
#
# Sharding: adj row-sharded across 8 cores ([2048, 16384] each), h replicated.
# Per-core algorithm (all in one pass over adj, memory-bound by the 128 MiB
# adj slice):
#   - DMA adj chunk [128, W] f32 -> SBUF
#   - ScalarE: E = exp(chunk) cast to f16 (SBUF->SBUF)
#   - TensorE: transpose 128x128 blocks of E via identity matmul -> PSUM
#   - VectorE/ScalarE (alternating): copy PSUM -> SBUF (still f16)
#   - TensorE: accumulate E_block^T.T @ h_aug into PSUM out tile, where
#     h_aug = [h | ones] f16 [128, 129]; column 128 accumulates the row sums
#     of exp(adj) so the softmax denominator comes free from the matmul.
#   - VectorE: out = psum[:, :128] * (1 / psum[:, 128]) -> DMA to HBM.

import os
import sys

for _p in ("/opt/trn_rl_repo", "/root/.axon_site/_ro/trn_rl_repo"):
    if os.path.isdir(_p) and _p not in sys.path:
        sys.path.append(_p)

import numpy as np

import concourse.bass as bass  # noqa: F401  (registers AP machinery)
import concourse.tile as tile
from concourse import bacc, mybir
from concourse.bass_utils import run_bass_kernel_spmd
from concourse.masks import make_identity

N_CORES = 8
N = 16384
D = 128
ROWS = N // N_CORES  # 2048 rows of adj per core
P = 128


def pack_h(h, d):
    """Host-side prep: [cols, d] f32 -> [128, cols//128, d+1] f32 with a ones
    column appended, laid out so h_aug[p, jc, :] = [h[jc*128 + p, :], 1.0].
    Contiguous in this layout so the device load is a single clean DMA."""
    cols = h.shape[0]
    njc = cols // P
    hp = h.reshape(njc, P, d).transpose(1, 0, 2)  # [p, jc, d]
    ones = np.ones((P, njc, 1), np.float32)
    return np.ascontiguousarray(np.concatenate([hp, ones], axis=2), dtype=np.float32)


def kernel_body(
    tc, out_ap, adj_ap, h_ap, rows, cols, d, w,
    adj_bufs=3, e_bufs=2, et_bufs=3, ps_bufs=2,
    grp=16, split_exp=False, copy_engines=("vector", "scalar"),
):
    """Tile kernel for one core: out = softmax_rows(adj) @ h.

    adj_ap: [rows, cols] f32 DRAM; h_ap: [128, cols//128, d+1] f32 DRAM
    (pre-packed by pack_h); out_ap: [rows, d] f32 DRAM.
    w: column chunk width (multiple of 128).
    """
    nc = tc.nc
    f32 = mybir.dt.float32
    f16 = mybir.dt.float16

    NT = rows // P  # row tiles
    CH = cols // w  # column chunks per row tile
    BPC = w // P  # 128-wide j-blocks per chunk
    GRP = min(grp, BPC)  # j-blocks per PSUM staging group
    NG = BPC // GRP
    NJC = cols // P  # total j-blocks
    assert rows % P == 0 and cols % w == 0 and w % (GRP * P) == 0
    copy_eng_idx = 0

    with (
        tc.tile_pool(name="singles", bufs=1) as singles,
        tc.tile_pool(name="adjp", bufs=adj_bufs) as adjp,
        tc.tile_pool(name="ep", bufs=e_bufs) as ep,
        tc.tile_pool(name="etp", bufs=et_bufs) as etp,
        tc.tile_pool(name="outp", bufs=2) as outp,
        tc.tile_pool(name="rp", bufs=2) as rp,
        tc.tile_pool(name="psacc", bufs=2, space="PSUM") as psacc,
        tc.tile_pool(name="psstage", bufs=ps_bufs, space="PSUM") as psstage,
    ):
        identity = singles.tile([P, P], f16)
        make_identity(nc, identity)

        # h_aug[p, jc, :] = [h[jc*128 + p, :], 1.0] in f16 (pre-packed on host)
        h_aug = singles.tile([P, NJC, d + 1], f16)
        nc.gpsimd.dma_start(out=h_aug[:], in_=h_ap)  # contiguous f32 -> f16 cast

        for it in range(NT):
            acc = psacc.tile([P, d + 1], f32)
            for c in range(CH):
                a = adjp.tile([P, w], f32)
                nc.sync.dma_start(a[:], adj_ap[it * P : (it + 1) * P, c * w : (c + 1) * w])
                e = ep.tile([P, w], f16)
                if split_exp:
                    gw = GRP * P
                    for g in range(NG):
                        nc.scalar.activation(
                            e[:, g * gw : (g + 1) * gw],
                            a[:, g * gw : (g + 1) * gw],
                            mybir.ActivationFunctionType.Exp,
                        )
                else:
                    nc.scalar.activation(e[:], a[:], mybir.ActivationFunctionType.Exp)
                for g in range(NG):
                    pt = psstage.tile([P, GRP, P], f16)
                    for q in range(GRP):
                        jb = g * GRP + q
                        nc.tensor.transpose(
                            pt[:, q, :], e[:, jb * P : (jb + 1) * P], identity
                        )
                    et = etp.tile([P, GRP, P], f16)
                    ceng = copy_engines[copy_eng_idx % len(copy_engines)]
                    copy_eng_idx += 1
                    if ceng == "scalar":
                        nc.scalar.copy(out=et[:], in_=pt[:])
                    else:
                        nc.vector.tensor_copy(out=et[:], in_=pt[:])
                    for q in range(GRP):
                        jc = c * BPC + g * GRP + q
                        nc.tensor.matmul(
                            acc[:],
                            lhsT=et[:, q, :],
                            rhs=h_aug[:, jc, :],
                            start=(jc == 0),
                            stop=(jc == NJC - 1),
                        )
            rec = rp.tile([P, 1], f32)
            nc.vector.reciprocal(rec[:], acc[:, d : d + 1])
            ob = outp.tile([P, d], f32)
            nc.vector.tensor_scalar_mul(ob[:], acc[:, 0:d], rec[:])
            nc.sync.dma_start(out_ap[it * P : (it + 1) * P, :], ob[:])


def build_nc(rows=ROWS, cols=N, d=D, w=4096, num_devices=N_CORES):
    nc = bacc.Bacc(
        "TRN2", target_bir_lowering=False, debug=False, num_devices=num_devices
    )
    adj = nc.dram_tensor("adj", [rows, cols], mybir.dt.float32, kind="ExternalInput").ap()
    h = nc.dram_tensor(
        "h", [P, cols // P, d + 1], mybir.dt.float32, kind="ExternalInput"
    ).ap()
    out = nc.dram_tensor("out", [rows, d], mybir.dt.float32, kind="ExternalOutput").ap()
    with tile.TileContext(nc) as tc:
        kernel_body(tc, out, adj, h, rows, cols, d, w)
    nc.compile()
    return nc


_nc_cache = None


def kernel(h, adj):
    global _nc_cache
    if _nc_cache is None:
        _nc_cache = build_nc()
    nc = _nc_cache
    h = np.ascontiguousarray(np.asarray(h, dtype=np.float32))
    adj = np.ascontiguousarray(np.asarray(adj, dtype=np.float32))
    h_packed = pack_h(h, D)
    in_maps = [
        {"adj": adj[i * ROWS : (i + 1) * ROWS], "h": h_packed}
        for i in range(N_CORES)
    ]
    res = run_bass_kernel_spmd(nc, in_maps, core_ids=list(range(N_CORES)))
    return np.concatenate([r["out"] for r in res.results], axis=0)
